# revision 12
# baseline (speedup 1.0000x reference)
"""BEV distillation mask generator (CenterPoint-style gaussian max-scatter) on TRN2.

Strategy (8 NeuronCores, data-parallel):
  core c handles frame c//2, box-half c%2 (1280 of 2560 boxes per frame).
  Per core:
    1. per-box params (radius bucket, cell, value) via DVE/ACT ops, boxes on
       partitions ([128, 10] tiles)
    2. exponential encoding u = 2^(K*v) (bf16): f32 PSUM *sum* over colliding
       boxes approximates *max* with error <= log2(n)/K (n-way value ties);
       data has only 2-way collisions -> error <= 1/120 ~ 0.008 << 2e-2 tol.
    3. scatter u to per-bucket point images via one-hot matmuls on PE
       (lhs = u * onehot(cy) bf16, rhs = onehot(128*bucketblk + cx) fp16)
    4. decode: v-hat = log2(max(S,1.1))/K via the float-bits trick (the Ln
       act table cannot digest S <= 2^121), then Ln(v-hat) + per-bucket
       2*sigma^2 scale -> scaled-log image, fp16
    5. per-bucket gaussian max-envelope = separable 2-pass shift-max DT in
       scaled-log space (additive -mag^2, bucket-independent); fp16
       tensor_scalar(4x)/tensor_tensor(2x) ops on DVE; Pool takes
       tensor_scalar/copy work (pyramid planes, clamps, some one-hots)
    6. exp per bucket (ACT), pairwise bucket max -> per-half [x, 2*128]
  Host combines half tiles, box-halves and frames with np.maximum and
  transposes to [y, x] (max-scatter is commutative) -> [4,1,128,128] f32.
"""
import numpy as np

FEAT = 128
NBOX = 1280          # boxes per core (half frame)
NT = NBOX // 128     # 10 box tiles
HALF_BUCKETS = [[9, 7, 5, 3], [8, 6, 4, 2]]   # balanced sum(b) = 24 / 20
K_ENC = 120.0
KLN2 = float(np.float32(K_ENC * np.log(2.0)))        # 83.17766
LN_BIAS = 1.1
LNEXP_SET_ID = 6     # act_info.json index of natural_log_exp_and_others

_prog_cache = {}


def _f(x):
    return float(np.float32(x))


def _build_program():
    import concourse.bass as bass
    import concourse.tile as tile
    from concourse import bacc, mybir

    dt = mybir.dt
    Alu = mybir.AluOpType
    Act = mybir.ActivationFunctionType

    nc = bacc.Bacc("TRN2", target_bir_lowering=False, debug=False, num_devices=8)

    par_d = nc.dram_tensor("par", [128, 6 * NT], dt.float32, kind="ExternalInput").ap()
    cst_d = nc.dram_tensor("cst", [128, 1152], dt.float16, kind="ExternalInput").ap()
    hm0_d = nc.dram_tensor("hm0", [128, 256], dt.float16, kind="ExternalOutput").ap()
    hm1_d = nc.dram_tensor("hm1", [128, 256], dt.float16, kind="ExternalOutput").ap()
    hm_out = [hm0_d, hm1_d]

    RECIP08 = _f(1.0 / np.float64(np.float32(0.8)))
    MAGIC = _f(8388608.0)

    def xp3(ap, dims, extra_off=0):
        return type(ap)(ap.tensor, ap.offset + extra_off, [ap.ap[0]] + dims)

    with tile.TileContext(nc) as tc:
        with (
            tc.tile_pool(name="const", bufs=1) as cpool,
            tc.tile_pool(name="par", bufs=1) as ppool,
            tc.tile_pool(name="mm", bufs=NT) as mpool,
            tc.tile_pool(name="dtw", bufs=1) as wpool,
        ):
            par = ppool.tile([128, 6 * NT], dt.float32, name="par")
            nc.sync.dma_start(par[:], par_d)
            cst = cpool.tile([128, 1152], dt.float16, name="cst")
            nc.sync.dma_start(cst[:, 0:1024], cst_d[:, 0:1024])
            nc.sync.dma_start(cst[:, 1024:1152], cst_d[:, 1024:1152])
            x = par[:, 0:NT]
            y = par[:, NT:2 * NT]
            wl = par[:, 2 * NT:4 * NT]
            sc = par[:, 4 * NT:5 * NT]
            cl = par[:, 5 * NT:6 * NT]
            iota1024 = cst[:, 0:1024]
            iota128 = cst[:, 0:128]
            ident16 = cst[:, 1024:1152]

            V = nc.vector    # DVE
            A = nc.scalar    # ACT
            G = nc.gpsimd    # Pool
            PE = nc.tensor

            _ptn = [0]

            def pt(shape=(128, NT), dtt=None, pool=ppool):
                _ptn[0] += 1
                return pool.tile(list(shape), dtt or dt.float32, name=f"pt{_ptn[0]}")

            def floor_(dst, src_ap, scr):
                V.tensor_scalar(dst, src_ap, MAGIC, MAGIC, Alu.add, Alu.subtract)
                V.tensor_tensor(scr, dst, src_ap, Alu.is_gt)
                V.tensor_tensor(dst, dst, scr, Alu.subtract)

            # ---- radius (batched over the 3 candidate formulas) ----
            wlf = pt((128, 2 * NT))
            V.tensor_scalar(wlf[:], wl, RECIP08, None, Alu.mult)
            w_fm, l_fm = wlf[:, 0:NT], wlf[:, NT:2 * NT]
            b1 = pt()
            V.tensor_tensor(b1[:], l_fm, w_fm, Alu.add)
            twh = pt()
            V.tensor_tensor(twh[:], w_fm, l_fm, Alu.mult)
            B3 = pt((128, 3 * NT))
            A.copy(B3[:, 0:NT], b1[:])
            A.mul(B3[:, NT:2 * NT], b1[:], _f(2.0))
            A.mul(B3[:, 2 * NT:3 * NT], b1[:], _f(-0.2))
            C3 = pt((128, 3 * NT))
            A.mul(C3[:, 0:NT], twh[:], _f(4.0 * 0.9 / 1.1))
            A.mul(C3[:, NT:2 * NT], twh[:], _f(16.0 * 0.9))
            A.mul(C3[:, 2 * NT:3 * NT], twh[:], _f(-16.0 * 0.1 * 0.9))
            D3 = pt((128, 3 * NT))
            V.tensor_tensor(D3[:], B3[:], B3[:], Alu.mult)
            V.tensor_tensor(D3[:], D3[:], C3[:], Alu.subtract)
            V.tensor_scalar(D3[:], D3[:], _f(0.0), None, Alu.max)
            A.activation(D3[:], D3[:], Act.Sqrt)
            R3 = pt((128, 3 * NT))
            V.tensor_tensor(R3[:], B3[:], D3[:], Alu.add)
            V.tensor_scalar(R3[:], R3[:], _f(0.5), None, Alu.mult)
            r = pt()
            V.tensor_tensor(r[:], R3[:, 0:NT], R3[:, NT:2 * NT], Alu.min)
            V.tensor_tensor(r[:], r[:], R3[:, 2 * NT:3 * NT], Alu.min)

            # ---- centers ----
            txv = pt(); V.tensor_scalar(txv[:], x, _f(51.2), RECIP08, Alu.add, Alu.mult)
            tyv = pt(); V.tensor_scalar(tyv[:], y, _f(51.2), RECIP08, Alu.add, Alu.mult)
            fscr = pt()
            cx = pt(); floor_(cx[:], txv[:], fscr[:])
            cy = pt(); floor_(cy[:], tyv[:], fscr[:])

            # ---- bucket block index + 1024-code ----
            fb = pt(); floor_(fb[:], r[:], fscr[:])
            bb = pt(); V.tensor_scalar(bb[:], fb[:], _f(2.0), _f(9.0), Alu.max, Alu.min)
            ee = pt(); V.tensor_scalar(ee[:], bb[:], _f(-1.0), _f(9.0), Alu.mult, Alu.add)
            eh = pt(); V.tensor_scalar(eh[:], ee[:], _f(0.5), None, Alu.mult)
            h2 = pt(); floor_(h2[:], eh[:], fscr[:])
            t2 = pt(); V.tensor_scalar(t2[:], h2[:], _f(2.0), None, Alu.mult)
            parb = pt(); V.tensor_tensor(parb[:], ee[:], t2[:], Alu.subtract)
            jh = pt(); V.tensor_scalar(jh[:], h2[:], _f(128.0), None, Alu.mult)
            jp = pt(); V.tensor_scalar(jp[:], parb[:], _f(512.0), None, Alu.mult)
            cstar = pt()
            V.tensor_tensor(cstar[:], jh[:], jp[:], Alu.add)
            V.tensor_tensor(cstar[:], cstar[:], cx[:], Alu.add)

            # ---- value ----
            ne7 = pt(); V.tensor_scalar(ne7[:], cl, _f(7.0), None, Alu.not_equal)
            sm = pt(); V.scalar_tensor_tensor(sm[:], cl, _f(5.0), ne7[:], Alu.is_ge, Alu.mult)
            vm = pt(); V.tensor_tensor(vm[:], sm[:], sc, Alu.mult)
            V.tensor_scalar(vm[:], vm[:], _f(0.5), _f(0.5), Alu.mult, Alu.add)
            mgf = pt(); V.tensor_scalar(mgf[:], cl, _f(0.0), None, Alu.is_ge)
            dv = pt()
            V.tensor_tensor(dv[:], vm[:], sc, Alu.subtract)
            V.tensor_tensor(dv[:], dv[:], mgf[:], Alu.mult)
            v = pt()
            V.tensor_tensor(v[:], sc, dv[:], Alu.add)

            # ---- encode u = exp(K*ln2 * v); pin the {Ln, Exp} act table ----
            A.add_instruction(mybir.InstLoadActFuncSet(
                name=nc.get_next_instruction_name(), act_func_set_id=LNEXP_SET_ID))
            u = pt()
            A.activation(u[:], v[:], Act.Exp, scale=KLN2)

            lnbias = cpool.tile([128, 1], dt.float32, name="lnbias")
            V.memset(lnbias[:], _f(-127.0 / K_ENC))

            # ---- one-hots + scatter matmuls (interleaved per tile) ----
            with (
                tc.tile_pool(name="psS", bufs=2, space="PSUM") as psS,
                tc.tile_pool(name="psT", bufs=2, space="PSUM") as psT,
            ):
                S = [psS.tile([128, 512], dt.float32, name=f"S{h}") for h in range(2)]
                for t in range(NT):
                    ey_eng = G if t >= 5 else V
                    ey_t = mpool.tile([128, 128], dt.bfloat16, name="ey")
                    ey_eng.tensor_scalar(ey_t[:], iota128, cy[:, t:t + 1], None,
                                         Alu.is_equal)
                    rhs_eng = G if t >= 8 else V
                    rhs_t = mpool.tile([128, 1024], dt.float16, name="rhs")
                    rhs_eng.tensor_scalar(rhs_t[:], iota1024, cstar[:, t:t + 1], None,
                                          Alu.is_equal)
                    lh_t = mpool.tile([128, 128], dt.bfloat16, name="lh")
                    A.mul(lh_t[:], ey_t[:], u[:, t:t + 1])
                    for h in range(2):
                        PE.matmul(S[h][:], lh_t[:], rhs_t[:, h * 512:(h + 1) * 512],
                                  start=(t == 0), stop=(t == NT - 1))

                # ---- decode both halves (Pool clamps, ACT Ln, inv_s scale) ----
                halves = []
                for h in range(2):
                    bks = HALF_BUCKETS[h]
                    Sb = wpool.tile([128, 512], dt.float32, name=f"Sb{h}")
                    V.tensor_scalar(Sb[:], S[h][:], _f(LN_BIAS), None, Alu.max)
                    Ln2 = wpool.tile([128, 512], dt.float16, name=f"Ln2_{h}")
                    A.activation(Ln2[:], Sb[:].bitcast(dt.int32), Act.Ln,
                                 scale=_f(2.0 ** -23 / K_ENC), bias=lnbias[:])
                    L = wpool.tile([128, 512], dt.float16, name=f"L{h}")
                    sc_eng = G if h == 1 else V
                    for j, b in enumerate(bks):
                        inv_s = _f((2 * b + 1) ** 2 / np.float32(18.0))
                        sc_eng.tensor_scalar(L[:, j * 128:(j + 1) * 128],
                                             Ln2[:, j * 128:(j + 1) * 128],
                                             inv_s, None, Alu.mult)
                    halves.append((bks, L))

                def dt_pass(h, src, pass_id):
                    """Shift-max DT along the free dim (fp16, single acc).
                    h==1: pyramid planes + acc init on Pool, maxes on DVE."""
                    bks, _ = halves[h]
                    bmax = bks[0]
                    nact = [sum(1 for b in bks if b >= m) for m in range(bmax + 1)]
                    aux = G if h == 1 else V
                    acc = wpool.tile([128, 512], dt.float16, name=f"acc{h}{pass_id}")
                    aux.tensor_copy(acc[:], src[:])
                    tmps = []
                    for m in range(1, bmax + 1):
                        n = nact[m]
                        tm = wpool.tile([128, 512], dt.float16,
                                        name=f"tmp{h}{pass_id}{m}")
                        tmps.append(tm)
                        aux.tensor_scalar(
                            xp3(tm[:], [[128, n], [1, 128]]),
                            xp3(src[:], [[128, n], [1, 128]]),
                            _f(-float(m * m)), None, Alu.add)
                    for m in range(1, bmax + 1):
                        n = nact[m]
                        tm = tmps[m - 1]
                        wlen = 128 - m
                        pv = xp3(acc[:], [[128, n], [1, wlen]], extra_off=m)
                        tv = xp3(tm[:], [[128, n], [1, wlen]])
                        V.tensor_tensor(pv, pv, tv, Alu.max)
                        nv = xp3(acc[:], [[128, n], [1, wlen]])
                        tv2 = xp3(tm[:], [[128, n], [1, wlen]], extra_off=m)
                        V.tensor_tensor(nv, nv, tv2, Alu.max)
                    return acc

                # pass-x h0 then h1 (h1 aux work rides on Pool meanwhile)
                accx = [None, None]
                accx[0] = dt_pass(0, halves[0][1], 0)
                accx[1] = dt_pass(1, halves[1][1], 0)

                accy = [None, None]
                for h in range(2):
                    Tp = psT.tile([128, 512], dt.float16, name=f"Tp{h}")
                    for j in range(4):
                        PE.transpose(Tp[:, j * 128:(j + 1) * 128],
                                     accx[h][:, j * 128:(j + 1) * 128], ident16)
                    L2 = wpool.tile([128, 512], dt.float16, name=f"L2_{h}")
                    V.tensor_copy(L2[:], Tp[:])
                    accy[h] = dt_pass(h, L2, 1)

                for h in range(2):
                    bks, _ = halves[h]
                    H = wpool.tile([128, 512], dt.float16, name=f"H{h}")
                    for j, b in enumerate(bks):
                        s_b = _f(np.float32(18.0) / (2 * b + 1) ** 2)
                        A.activation(H[:, j * 128:(j + 1) * 128],
                                     accy[h][:, j * 128:(j + 1) * 128],
                                     Act.Exp, scale=s_b)
                    hred = wpool.tile([128, 256], dt.float16, name=f"hred{h}")
                    V.tensor_tensor(hred[:], H[:, 0:256], H[:, 256:512], Alu.max)
                    nc.sync.dma_start(hm_out[h], hred[:])

    nc.compile()
    return nc


def _consts():
    iota1024 = np.arange(1024, dtype=np.float16)
    cst = np.concatenate([
        np.broadcast_to(iota1024, (128, 1024)),
        np.eye(128, dtype=np.float16),
    ], axis=1)
    return np.ascontiguousarray(cst)


def _shard_inputs(refined_rois, refined_scores, medium_gts, medium_scores,
                  near_unmatched, medium_unmatched):
    """Build the 8 per-core input maps (pure layout/sharding, no math)."""
    cst = _consts()
    in_maps = []
    B = refined_rois.shape[0]
    n_rr = refined_rois.shape[1]
    n_nu = near_unmatched.shape[1]
    n_mu = medium_unmatched.shape[1]
    for f in range(B):
        bx = np.concatenate([refined_rois[f][:, :7], medium_gts[f][:, :7],
                             near_unmatched[f][:, :7], medium_unmatched[f][:, :7]], 0)
        score = np.concatenate([refined_scores[f], medium_scores[f],
                                np.full(n_nu, 0.4, np.float32),
                                np.full(n_mu, 0.2, np.float32)])
        cls = np.concatenate([np.full(n_rr, -1.0, np.float32), medium_gts[f][:, 7],
                              np.full(n_nu, -1.0, np.float32),
                              np.full(n_mu, -1.0, np.float32)])
        for hf in range(2):
            sl = slice(hf * NBOX, (hf + 1) * NBOX)

            def lay(a):
                return a[sl].astype(np.float32).reshape(NT, 128).T

            par = np.concatenate([lay(bx[:, 0]), lay(bx[:, 1]), lay(bx[:, 3]),
                                  lay(bx[:, 4]), lay(score), lay(cls)], axis=1)
            in_maps.append(dict(par=np.ascontiguousarray(par), cst=cst))
    return in_maps


def kernel(**inputs) -> np.ndarray:
    from concourse.bass_utils import run_bass_kernel_spmd

    if "nc" not in _prog_cache:
        _prog_cache["nc"] = _build_program()
    nc = _prog_cache["nc"]

    in_maps = _shard_inputs(**{k: np.asarray(v) for k, v in inputs.items()})
    res = run_bass_kernel_spmd(nc, in_maps, core_ids=list(range(8)))
    B = np.asarray(inputs["refined_rois"]).shape[0]
    out = np.empty((B, 1, FEAT, FEAT), np.float32)
    for f in range(B):
        m = None
        for c in (2 * f, 2 * f + 1):
            for k in ("hm0", "hm1"):
                t = res.results[c][k]
                q = np.maximum(t[:, 0:128], t[:, 128:256])
                m = q if m is None else np.maximum(m, q)
        out[f, 0] = m.astype(np.float32).T
    return out


# revision 13
# speedup vs baseline: 1.0207x; 1.0207x over previous
"""BEV distillation mask generator (CenterPoint-style gaussian max-scatter) on TRN2.

Strategy (8 NeuronCores, data-parallel):
  core c handles frame c//2, box-half c%2 (1280 of 2560 boxes per frame).
  Per core:
    1. per-box params (radius bucket, cell, value) via DVE/ACT ops, boxes on
       partitions ([128, 10] tiles)
    2. exponential encoding u = 2^(K*v) (bf16): f32 PSUM *sum* over colliding
       boxes approximates *max* with error <= log2(n)/K (n-way value ties);
       data has only 2-way collisions -> error <= 1/120 ~ 0.008 << 2e-2 tol.
    3. scatter u to per-bucket point images via one-hot matmuls on PE
       (lhs = u * onehot(cy) bf16, rhs = onehot(128*bucketblk + cx) fp16)
    4. decode: v-hat = log2(max(S,1.1))/K via the float-bits trick (the Ln
       act table cannot digest S <= 2^121), then Ln(v-hat) + per-bucket
       2*sigma^2 scale -> scaled-log image, fp16
    5. per-bucket gaussian max-envelope = separable 2-pass shift-max DT in
       scaled-log space (additive -mag^2, bucket-independent); fp16
       tensor_scalar(4x)/tensor_tensor(2x) ops on DVE; Pool takes
       tensor_scalar/copy work (pyramid planes, clamps, some one-hots)
    6. exp per bucket (ACT), pairwise bucket max -> per-half [x, 2*128]
  Host combines half tiles, box-halves and frames with np.maximum and
  transposes to [y, x] (max-scatter is commutative) -> [4,1,128,128] f32.
"""
import numpy as np

FEAT = 128
NBOX = 1280          # boxes per core (half frame)
NT = NBOX // 128     # 10 box tiles
HALF_BUCKETS = [[9, 7, 5, 3], [8, 6, 4, 2]]   # balanced sum(b) = 24 / 20
K_ENC = 120.0
KLN2 = float(np.float32(K_ENC * np.log(2.0)))        # 83.17766
LN_BIAS = 1.1
LNEXP_SET_ID = 6     # act_info.json index of natural_log_exp_and_others

_prog_cache = {}


def _f(x):
    return float(np.float32(x))


def _build_program():
    import concourse.bass as bass
    import concourse.tile as tile
    from concourse import bacc, mybir

    dt = mybir.dt
    Alu = mybir.AluOpType
    Act = mybir.ActivationFunctionType

    nc = bacc.Bacc("TRN2", target_bir_lowering=False, debug=False, num_devices=8)

    par_d = nc.dram_tensor("par", [128, 6 * NT], dt.float32, kind="ExternalInput").ap()
    cst_d = nc.dram_tensor("cst", [128, 1152], dt.float16, kind="ExternalInput").ap()
    hm0_d = nc.dram_tensor("hm0", [128, 256], dt.float16, kind="ExternalOutput").ap()
    hm1_d = nc.dram_tensor("hm1", [128, 256], dt.float16, kind="ExternalOutput").ap()
    hm_out = [hm0_d, hm1_d]

    RECIP08 = _f(1.0 / np.float64(np.float32(0.8)))
    MAGIC = _f(8388608.0)

    def xp3(ap, dims, extra_off=0):
        return type(ap)(ap.tensor, ap.offset + extra_off, [ap.ap[0]] + dims)

    with tile.TileContext(nc) as tc:
        with (
            tc.tile_pool(name="const", bufs=1) as cpool,
            tc.tile_pool(name="par", bufs=1) as ppool,
            tc.tile_pool(name="mm", bufs=NT) as mpool,
            tc.tile_pool(name="dtw", bufs=1) as wpool,
        ):
            par = ppool.tile([128, 6 * NT], dt.float32, name="par")
            nc.sync.dma_start(par[:], par_d)
            cst = cpool.tile([128, 1152], dt.float16, name="cst")
            nc.sync.dma_start(cst[:, 0:1024], cst_d[:, 0:1024])
            nc.sync.dma_start(cst[:, 1024:1152], cst_d[:, 1024:1152])
            x = par[:, 0:NT]
            y = par[:, NT:2 * NT]
            wl = par[:, 2 * NT:4 * NT]
            sc = par[:, 4 * NT:5 * NT]
            cl = par[:, 5 * NT:6 * NT]
            iota1024 = cst[:, 0:1024]
            iota128 = cst[:, 0:128]
            ident16 = cst[:, 1024:1152]

            V = nc.vector    # DVE
            A = nc.scalar    # ACT
            G = nc.gpsimd    # Pool
            PE = nc.tensor

            _ptn = [0]

            def pt(shape=(128, NT), dtt=None, pool=ppool):
                _ptn[0] += 1
                return pool.tile(list(shape), dtt or dt.float32, name=f"pt{_ptn[0]}")

            def floor_(dst, src_ap, scr):
                V.tensor_scalar(dst, src_ap, MAGIC, MAGIC, Alu.add, Alu.subtract)
                V.tensor_tensor(scr, dst, src_ap, Alu.is_gt)
                V.tensor_tensor(dst, dst, scr, Alu.subtract)

            # ---- radius (batched over the 3 candidate formulas) ----
            wlf = pt((128, 2 * NT))
            V.tensor_scalar(wlf[:], wl, RECIP08, None, Alu.mult)
            w_fm, l_fm = wlf[:, 0:NT], wlf[:, NT:2 * NT]
            b1 = pt()
            V.tensor_tensor(b1[:], l_fm, w_fm, Alu.add)
            twh = pt()
            V.tensor_tensor(twh[:], w_fm, l_fm, Alu.mult)
            B3 = pt((128, 3 * NT))
            A.copy(B3[:, 0:NT], b1[:])
            A.mul(B3[:, NT:2 * NT], b1[:], _f(2.0))
            A.mul(B3[:, 2 * NT:3 * NT], b1[:], _f(-0.2))
            C3 = pt((128, 3 * NT))
            A.mul(C3[:, 0:NT], twh[:], _f(4.0 * 0.9 / 1.1))
            A.mul(C3[:, NT:2 * NT], twh[:], _f(16.0 * 0.9))
            A.mul(C3[:, 2 * NT:3 * NT], twh[:], _f(-16.0 * 0.1 * 0.9))
            D3 = pt((128, 3 * NT))
            V.tensor_tensor(D3[:], B3[:], B3[:], Alu.mult)
            V.tensor_tensor(D3[:], D3[:], C3[:], Alu.subtract)
            V.tensor_scalar(D3[:], D3[:], _f(0.0), None, Alu.max)
            A.activation(D3[:], D3[:], Act.Sqrt)
            R3 = pt((128, 3 * NT))
            V.tensor_tensor(R3[:], B3[:], D3[:], Alu.add)
            V.tensor_scalar(R3[:], R3[:], _f(0.5), None, Alu.mult)
            r = pt()
            V.tensor_tensor(r[:], R3[:, 0:NT], R3[:, NT:2 * NT], Alu.min)
            V.tensor_tensor(r[:], r[:], R3[:, 2 * NT:3 * NT], Alu.min)

            # ---- centers ----
            txv = pt(); V.tensor_scalar(txv[:], x, _f(51.2), RECIP08, Alu.add, Alu.mult)
            tyv = pt(); V.tensor_scalar(tyv[:], y, _f(51.2), RECIP08, Alu.add, Alu.mult)
            fscr = pt()
            cx = pt(); floor_(cx[:], txv[:], fscr[:])
            cy = pt(); floor_(cy[:], tyv[:], fscr[:])

            # ---- bucket block index + 1024-code ----
            fb = pt(); floor_(fb[:], r[:], fscr[:])
            bb = pt(); V.tensor_scalar(bb[:], fb[:], _f(2.0), _f(9.0), Alu.max, Alu.min)
            ee = pt(); V.tensor_scalar(ee[:], bb[:], _f(-1.0), _f(9.0), Alu.mult, Alu.add)
            eh = pt(); V.tensor_scalar(eh[:], ee[:], _f(0.5), None, Alu.mult)
            h2 = pt(); floor_(h2[:], eh[:], fscr[:])
            t2 = pt(); V.tensor_scalar(t2[:], h2[:], _f(2.0), None, Alu.mult)
            parb = pt(); V.tensor_tensor(parb[:], ee[:], t2[:], Alu.subtract)
            jh = pt(); V.tensor_scalar(jh[:], h2[:], _f(128.0), None, Alu.mult)
            jp = pt(); V.tensor_scalar(jp[:], parb[:], _f(512.0), None, Alu.mult)
            cstar = pt()
            V.tensor_tensor(cstar[:], jh[:], jp[:], Alu.add)
            V.tensor_tensor(cstar[:], cstar[:], cx[:], Alu.add)

            # ---- value ----
            ne7 = pt(); V.tensor_scalar(ne7[:], cl, _f(7.0), None, Alu.not_equal)
            sm = pt(); V.scalar_tensor_tensor(sm[:], cl, _f(5.0), ne7[:], Alu.is_ge, Alu.mult)
            vm = pt(); V.tensor_tensor(vm[:], sm[:], sc, Alu.mult)
            V.tensor_scalar(vm[:], vm[:], _f(0.5), _f(0.5), Alu.mult, Alu.add)
            mgf = pt(); V.tensor_scalar(mgf[:], cl, _f(0.0), None, Alu.is_ge)
            dv = pt()
            V.tensor_tensor(dv[:], vm[:], sc, Alu.subtract)
            V.tensor_tensor(dv[:], dv[:], mgf[:], Alu.mult)
            v = pt()
            V.tensor_tensor(v[:], sc, dv[:], Alu.add)

            # ---- encode u = exp(K*ln2 * v); pin the {Ln, Exp} act table ----
            A.add_instruction(mybir.InstLoadActFuncSet(
                name=nc.get_next_instruction_name(), act_func_set_id=LNEXP_SET_ID))
            u = pt()
            A.activation(u[:], v[:], Act.Exp, scale=KLN2)

            lnbias = cpool.tile([128, 1], dt.float32, name="lnbias")
            V.memset(lnbias[:], _f(-127.0 / K_ENC))

            # ---- one-hots + scatter matmuls (interleaved per tile) ----
            with (
                tc.tile_pool(name="psS", bufs=2, space="PSUM") as psS,
                tc.tile_pool(name="psT", bufs=2, space="PSUM") as psT,
            ):
                S = [psS.tile([128, 512], dt.float32, name=f"S{h}") for h in range(2)]
                tiles = []
                for t in range(NT):
                    ey_t = mpool.tile([128, 128], dt.bfloat16, name="ey")
                    V.tensor_scalar(ey_t[:], iota128, cy[:, t:t + 1], None,
                                    Alu.is_equal)
                    rhs_eng = G if t >= 8 else V
                    rhs_t = mpool.tile([128, 1024], dt.float16, name="rhs")
                    rhs_eng.tensor_scalar(rhs_t[:], iota1024, cstar[:, t:t + 1], None,
                                          Alu.is_equal)
                    lh_t = mpool.tile([128, 128], dt.bfloat16, name="lh")
                    A.mul(lh_t[:], ey_t[:], u[:, t:t + 1])
                    tiles.append((lh_t, rhs_t))
                    PE.matmul(S[0][:], lh_t[:], rhs_t[:, 0:512],
                              start=(t == 0), stop=(t == NT - 1))
                for t in range(NT):
                    PE.matmul(S[1][:], tiles[t][0][:], tiles[t][1][:, 512:1024],
                              start=(t == 0), stop=(t == NT - 1))

                NACT = [[sum(1 for b in bks if b >= m) for m in range(bks[0] + 1)]
                        for bks in HALF_BUCKETS]

                def decode(h, sc_eng):
                    Sb = wpool.tile([128, 512], dt.float32, name=f"Sb{h}")
                    V.tensor_scalar(Sb[:], S[h][:], _f(LN_BIAS), None, Alu.max)
                    Ln2 = wpool.tile([128, 512], dt.float16, name=f"Ln2_{h}")
                    A.activation(Ln2[:], Sb[:].bitcast(dt.int32), Act.Ln,
                                 scale=_f(2.0 ** -23 / K_ENC), bias=lnbias[:])
                    L = wpool.tile([128, 512], dt.float16, name=f"L{h}")
                    for j, b in enumerate(HALF_BUCKETS[h]):
                        inv_s = _f((2 * b + 1) ** 2 / np.float32(18.0))
                        sc_eng.tensor_scalar(L[:, j * 128:(j + 1) * 128],
                                             Ln2[:, j * 128:(j + 1) * 128],
                                             inv_s, None, Alu.mult)
                    return L

                def dt_pyramid(h, src, pass_id, eng):
                    """acc copy + bias planes tmp[m] = src - m*m on `eng`."""
                    bmax = HALF_BUCKETS[h][0]
                    nact = NACT[h]
                    acc = wpool.tile([128, 512], dt.float16, name=f"acc{h}{pass_id}")
                    eng.tensor_copy(acc[:], src[:])
                    tmps = []
                    for m in range(1, bmax + 1):
                        n = nact[m]
                        tm = wpool.tile([128, 512], dt.float16,
                                        name=f"tmp{h}{pass_id}{m}")
                        tmps.append(tm)
                        eng.tensor_scalar(
                            xp3(tm[:], [[128, n], [1, 128]]),
                            xp3(src[:], [[128, n], [1, 128]]),
                            _f(-float(m * m)), None, Alu.add)
                    return acc, tmps

                def dt_chain(h, acc, tmps):
                    """Serial shift-max accumulation on DVE (fp16 2x)."""
                    bmax = HALF_BUCKETS[h][0]
                    nact = NACT[h]
                    for m in range(1, bmax + 1):
                        n = nact[m]
                        tm = tmps[m - 1]
                        wlen = 128 - m
                        pv = xp3(acc[:], [[128, n], [1, wlen]], extra_off=m)
                        tv = xp3(tm[:], [[128, n], [1, wlen]])
                        V.tensor_tensor(pv, pv, tv, Alu.max)
                        nv = xp3(acc[:], [[128, n], [1, wlen]])
                        tv2 = xp3(tm[:], [[128, n], [1, wlen]], extra_off=m)
                        V.tensor_tensor(nv, nv, tv2, Alu.max)
                    return acc

                def transpose_half(h, accx):
                    Tp = psT.tile([128, 512], dt.float16, name=f"Tp{h}")
                    for j in range(4):
                        PE.transpose(Tp[:, j * 128:(j + 1) * 128],
                                     accx[:, j * 128:(j + 1) * 128], ident16)
                    L2 = wpool.tile([128, 512], dt.float16, name=f"L2_{h}")
                    V.tensor_copy(L2[:], Tp[:])
                    return L2

                def heat_out(h, accy):
                    bks = HALF_BUCKETS[h]
                    H = wpool.tile([128, 512], dt.float16, name=f"H{h}")
                    for j, b in enumerate(bks):
                        s_b = _f(np.float32(18.0) / (2 * b + 1) ** 2)
                        A.activation(H[:, j * 128:(j + 1) * 128],
                                     accy[:, j * 128:(j + 1) * 128],
                                     Act.Exp, scale=s_b)
                    hred = wpool.tile([128, 256], dt.float16, name=f"hred{h}")
                    V.tensor_tensor(hred[:], H[:, 0:256], H[:, 256:512], Alu.max)
                    nc.sync.dma_start(hm_out[h], hred[:])

                # hand-scheduled emission: h1 aux work on Pool rides alongside
                # the DVE-owned h0 chains; Sb1 issued between pyramid and chain
                # so the DVE queue reaches it right as the S1 matmuls close.
                L0 = decode(0, V)
                a0x, t0x = dt_pyramid(0, L0, 0, V)
                L1 = decode(1, G)
                a1x, t1x = dt_pyramid(1, L1, 0, G)
                dt_chain(0, a0x, t0x)
                L2_0 = transpose_half(0, a0x)
                dt_chain(1, a1x, t1x)
                L2_1 = transpose_half(1, a1x)
                a0y, t0y = dt_pyramid(0, L2_0, 1, V)
                a1y, t1y = dt_pyramid(1, L2_1, 1, G)
                dt_chain(0, a0y, t0y)
                heat_out(0, a0y)
                dt_chain(1, a1y, t1y)
                heat_out(1, a1y)

    nc.compile()
    return nc


def _consts():
    iota1024 = np.arange(1024, dtype=np.float16)
    cst = np.concatenate([
        np.broadcast_to(iota1024, (128, 1024)),
        np.eye(128, dtype=np.float16),
    ], axis=1)
    return np.ascontiguousarray(cst)


def _shard_inputs(refined_rois, refined_scores, medium_gts, medium_scores,
                  near_unmatched, medium_unmatched):
    """Build the 8 per-core input maps (pure layout/sharding, no math)."""
    cst = _consts()
    in_maps = []
    B = refined_rois.shape[0]
    n_rr = refined_rois.shape[1]
    n_nu = near_unmatched.shape[1]
    n_mu = medium_unmatched.shape[1]
    for f in range(B):
        bx = np.concatenate([refined_rois[f][:, :7], medium_gts[f][:, :7],
                             near_unmatched[f][:, :7], medium_unmatched[f][:, :7]], 0)
        score = np.concatenate([refined_scores[f], medium_scores[f],
                                np.full(n_nu, 0.4, np.float32),
                                np.full(n_mu, 0.2, np.float32)])
        cls = np.concatenate([np.full(n_rr, -1.0, np.float32), medium_gts[f][:, 7],
                              np.full(n_nu, -1.0, np.float32),
                              np.full(n_mu, -1.0, np.float32)])
        for hf in range(2):
            sl = slice(hf * NBOX, (hf + 1) * NBOX)

            def lay(a):
                return a[sl].astype(np.float32).reshape(NT, 128).T

            par = np.concatenate([lay(bx[:, 0]), lay(bx[:, 1]), lay(bx[:, 3]),
                                  lay(bx[:, 4]), lay(score), lay(cls)], axis=1)
            in_maps.append(dict(par=np.ascontiguousarray(par), cst=cst))
    return in_maps


def kernel(**inputs) -> np.ndarray:
    from concourse.bass_utils import run_bass_kernel_spmd

    if "nc" not in _prog_cache:
        _prog_cache["nc"] = _build_program()
    nc = _prog_cache["nc"]

    in_maps = _shard_inputs(**{k: np.asarray(v) for k, v in inputs.items()})
    res = run_bass_kernel_spmd(nc, in_maps, core_ids=list(range(8)))
    B = np.asarray(inputs["refined_rois"]).shape[0]
    out = np.empty((B, 1, FEAT, FEAT), np.float32)
    for f in range(B):
        m = None
        for c in (2 * f, 2 * f + 1):
            for k in ("hm0", "hm1"):
                t = res.results[c][k]
                q = np.maximum(t[:, 0:128], t[:, 128:256])
                m = q if m is None else np.maximum(m, q)
        out[f, 0] = m.astype(np.float32).T
    return out


# revision 14
# speedup vs baseline: 1.1421x; 1.1189x over previous
"""BEV distillation mask generator (CenterPoint-style gaussian max-scatter) on TRN2.

Strategy (8 NeuronCores, data-parallel):
  core c handles frame c//2, box-half c%2 (1280 of 2560 boxes per frame).
  Per core:
    1. per-box params (radius bucket, cell, value) via DVE/ACT ops, boxes on
       partitions ([128, 10] tiles)
    2. exponential encoding u = 2^(K*v) (bf16): f32 PSUM *sum* over colliding
       boxes approximates *max* with error <= log2(n)/K (n-way value ties);
       data has only 2-way collisions -> error <= 1/120 ~ 0.008 << 2e-2 tol.
    3. scatter u to per-bucket point images via one-hot matmuls on PE
       (lhs = u * onehot(cy) bf16, rhs = onehot(128*bucketblk + cx) fp16)
    4. decode: v-hat = log2(max(S,1.1))/K via the float-bits trick (the Ln
       act table cannot digest S <= 2^121), then Ln(v-hat) + per-bucket
       2*sigma^2 scale -> scaled-log image, fp16
    5. per-bucket gaussian max-envelope = separable 2-pass shift-max DT in
       scaled-log space (additive -mag^2, bucket-independent); fp16
       tensor_scalar(4x)/tensor_tensor(2x) ops on DVE; Pool takes
       tensor_scalar/copy work (pyramid planes, clamps, some one-hots)
    6. exp per bucket (ACT), pairwise bucket max -> per-half [x, 2*128]
  Host combines half tiles, box-halves and frames with np.maximum and
  transposes to [y, x] (max-scatter is commutative) -> [4,1,128,128] f32.
"""
import numpy as np

FEAT = 128
NBOX = 1280          # boxes per core (half frame)
NT = NBOX // 128     # 10 box tiles
HALF_BUCKETS = [[9, 7, 5, 3], [8, 6, 4, 2]]   # balanced sum(b) = 24 / 20
K_ENC = 120.0
KLN2 = float(np.float32(K_ENC * np.log(2.0)))        # 83.17766
LN_BIAS = 1.1
LNEXP_SET_ID = 6     # act_info.json index of natural_log_exp_and_others

_prog_cache = {}


def _f(x):
    return float(np.float32(x))


def _build_program():
    import concourse.bass as bass
    import concourse.tile as tile
    from concourse import bacc, mybir

    dt = mybir.dt
    Alu = mybir.AluOpType
    Act = mybir.ActivationFunctionType

    nc = bacc.Bacc("TRN2", target_bir_lowering=False, debug=False, num_devices=8)

    par_d = nc.dram_tensor("par", [128, 6 * NT], dt.float32, kind="ExternalInput").ap()
    cst_d = nc.dram_tensor("cst", [128, 1152], dt.float16, kind="ExternalInput").ap()
    hm0_d = nc.dram_tensor("hm0", [128, 256], dt.float16, kind="ExternalOutput").ap()
    hm1_d = nc.dram_tensor("hm1", [128, 256], dt.float16, kind="ExternalOutput").ap()
    hm_out = [hm0_d, hm1_d]

    RECIP08 = _f(1.0 / np.float64(np.float32(0.8)))
    MAGIC = _f(8388608.0)

    def xp3(ap, dims, extra_off=0):
        return type(ap)(ap.tensor, ap.offset + extra_off, [ap.ap[0]] + dims)

    with tile.TileContext(nc) as tc:
        with (
            tc.tile_pool(name="const", bufs=1) as cpool,
            tc.tile_pool(name="par", bufs=1) as ppool,
            tc.tile_pool(name="mm", bufs=NT) as mpool,
            tc.tile_pool(name="dtw", bufs=1) as wpool,
        ):
            par = ppool.tile([128, 6 * NT], dt.float32, name="par")
            nc.sync.dma_start(par[:], par_d)
            cst = cpool.tile([128, 1152], dt.float16, name="cst")
            nc.sync.dma_start(cst[:, 0:1024], cst_d[:, 0:1024])
            nc.sync.dma_start(cst[:, 1024:1152], cst_d[:, 1024:1152])
            x = par[:, 0:NT]
            y = par[:, NT:2 * NT]
            wl = par[:, 2 * NT:4 * NT]
            sc = par[:, 4 * NT:5 * NT]
            cl = par[:, 5 * NT:6 * NT]
            iota1024 = cst[:, 0:1024]
            iota128 = cst[:, 0:128]
            ident16 = cst[:, 1024:1152]

            V = nc.vector    # DVE
            A = nc.scalar    # ACT
            G = nc.gpsimd    # Pool
            PE = nc.tensor

            _ptn = [0]

            def pt(shape=(128, NT), dtt=None, pool=ppool):
                _ptn[0] += 1
                return pool.tile(list(shape), dtt or dt.float32, name=f"pt{_ptn[0]}")

            def floor_(dst, src_ap, scr):
                V.tensor_scalar(dst, src_ap, MAGIC, MAGIC, Alu.add, Alu.subtract)
                V.tensor_tensor(scr, dst, src_ap, Alu.is_gt)
                V.tensor_tensor(dst, dst, scr, Alu.subtract)

            # ---- radius (batched over the 3 candidate formulas) ----
            wlf = pt((128, 2 * NT))
            V.tensor_scalar(wlf[:], wl, RECIP08, None, Alu.mult)
            w_fm, l_fm = wlf[:, 0:NT], wlf[:, NT:2 * NT]
            b1 = pt()
            V.tensor_tensor(b1[:], l_fm, w_fm, Alu.add)
            twh = pt()
            V.tensor_tensor(twh[:], w_fm, l_fm, Alu.mult)
            B3 = pt((128, 3 * NT))
            V.tensor_copy(B3[:, 0:NT], b1[:])
            V.tensor_scalar(B3[:, NT:2 * NT], b1[:], _f(2.0), None, Alu.mult)
            V.tensor_scalar(B3[:, 2 * NT:3 * NT], b1[:], _f(-0.2), None, Alu.mult)
            C3 = pt((128, 3 * NT))
            V.tensor_scalar(C3[:, 0:NT], twh[:], _f(4.0 * 0.9 / 1.1), None, Alu.mult)
            V.tensor_scalar(C3[:, NT:2 * NT], twh[:], _f(16.0 * 0.9), None, Alu.mult)
            V.tensor_scalar(C3[:, 2 * NT:3 * NT], twh[:], _f(-16.0 * 0.1 * 0.9), None, Alu.mult)
            D3 = pt((128, 3 * NT))
            V.tensor_tensor(D3[:], B3[:], B3[:], Alu.mult)
            V.tensor_tensor(D3[:], D3[:], C3[:], Alu.subtract)
            V.tensor_scalar(D3[:], D3[:], _f(0.0), None, Alu.max)
            A.activation(D3[:], D3[:], Act.Sqrt)
            R3 = pt((128, 3 * NT))
            V.tensor_tensor(R3[:], B3[:], D3[:], Alu.add)
            V.tensor_scalar(R3[:], R3[:], _f(0.5), None, Alu.mult)
            r = pt()
            V.tensor_tensor(r[:], R3[:, 0:NT], R3[:, NT:2 * NT], Alu.min)
            V.tensor_tensor(r[:], r[:], R3[:, 2 * NT:3 * NT], Alu.min)

            # ---- centers ----
            txv = pt(); V.tensor_scalar(txv[:], x, _f(51.2), RECIP08, Alu.add, Alu.mult)
            tyv = pt(); V.tensor_scalar(tyv[:], y, _f(51.2), RECIP08, Alu.add, Alu.mult)
            fscr = pt()
            cx = pt(); floor_(cx[:], txv[:], fscr[:])
            cy = pt(); floor_(cy[:], tyv[:], fscr[:])

            # ---- bucket block index + 1024-code ----
            fb = pt(); floor_(fb[:], r[:], fscr[:])
            bb = pt(); V.tensor_scalar(bb[:], fb[:], _f(2.0), _f(9.0), Alu.max, Alu.min)
            ee = pt(); V.tensor_scalar(ee[:], bb[:], _f(-1.0), _f(9.0), Alu.mult, Alu.add)
            eh = pt(); V.tensor_scalar(eh[:], ee[:], _f(0.5), None, Alu.mult)
            h2 = pt(); floor_(h2[:], eh[:], fscr[:])
            t2 = pt(); V.tensor_scalar(t2[:], h2[:], _f(2.0), None, Alu.mult)
            parb = pt(); V.tensor_tensor(parb[:], ee[:], t2[:], Alu.subtract)
            jh = pt(); V.tensor_scalar(jh[:], h2[:], _f(128.0), None, Alu.mult)
            jp = pt(); V.tensor_scalar(jp[:], parb[:], _f(512.0), None, Alu.mult)
            cstar = pt()
            V.tensor_tensor(cstar[:], jh[:], jp[:], Alu.add)
            V.tensor_tensor(cstar[:], cstar[:], cx[:], Alu.add)

            # Pool builds the last two rhs one-hots (emitted right after
            # cstar so its engine-level sync clears early)
            pool_rhs = {}
            for t in (8, 9):
                rhs_t = mpool.tile([128, 1024], dt.float16, name=f"rhsp{t}")
                G.tensor_scalar(rhs_t[:], iota1024, cstar[:, t:t + 1], None,
                                Alu.is_equal)
                pool_rhs[t] = rhs_t

            # ---- value ----
            ne7 = pt(); V.tensor_scalar(ne7[:], cl, _f(7.0), None, Alu.not_equal)
            sm = pt(); V.scalar_tensor_tensor(sm[:], cl, _f(5.0), ne7[:], Alu.is_ge, Alu.mult)
            vm = pt(); V.tensor_tensor(vm[:], sm[:], sc, Alu.mult)
            V.tensor_scalar(vm[:], vm[:], _f(0.5), _f(0.5), Alu.mult, Alu.add)
            mgf = pt(); V.tensor_scalar(mgf[:], cl, _f(0.0), None, Alu.is_ge)
            dv = pt()
            V.tensor_tensor(dv[:], vm[:], sc, Alu.subtract)
            V.tensor_tensor(dv[:], dv[:], mgf[:], Alu.mult)
            v = pt()
            V.tensor_tensor(v[:], sc, dv[:], Alu.add)

            # ---- encode u ~= 2^(K*v) via the inverse float-bits trick: the
            # decode is the same linear bits<->log2 map, so singleton cells
            # round-trip exactly; no ACT Exp (keeps ACT off the matmul gate).
            qf = pt()
            V.tensor_scalar(qf[:], v[:], _f(K_ENC * 8388608.0),
                            _f(127.0 * 8388608.0), Alu.mult, Alu.add)
            qi = pt(dtt=dt.int32)
            V.tensor_copy(qi[:], qf[:])
            u = qi[:].bitcast(dt.float32)

            lnbias = cpool.tile([128, 1], dt.float32, name="lnbias")
            V.memset(lnbias[:], _f(-127.0 / K_ENC))

            # ---- one-hots + scatter matmuls (interleaved per tile) ----
            with (
                tc.tile_pool(name="psS", bufs=2, space="PSUM") as psS,
                tc.tile_pool(name="psT", bufs=2, space="PSUM") as psT,
            ):
                S = [psS.tile([128, 512], dt.float32, name=f"S{h}") for h in range(2)]
                tiles = []
                for t in range(NT):
                    ey_t = mpool.tile([128, 128], dt.bfloat16, name="ey")
                    V.tensor_scalar(ey_t[:], iota128, cy[:, t:t + 1], None,
                                    Alu.is_equal)
                    if t in pool_rhs:
                        rhs_t = pool_rhs[t]
                    else:
                        rhs_t = mpool.tile([128, 1024], dt.float16, name="rhs")
                        V.tensor_scalar(rhs_t[:], iota1024, cstar[:, t:t + 1], None,
                                        Alu.is_equal)
                    lh_t = mpool.tile([128, 128], dt.bfloat16, name="lh")
                    A.mul(lh_t[:], ey_t[:], u[:, t:t + 1])
                    tiles.append((lh_t, rhs_t))
                    PE.matmul(S[0][:], lh_t[:], rhs_t[:, 0:512],
                              start=(t == 0), stop=(t == NT - 1))
                A.add_instruction(mybir.InstLoadActFuncSet(
                    name=nc.get_next_instruction_name(),
                    act_func_set_id=LNEXP_SET_ID))
                for t in range(NT):
                    PE.matmul(S[1][:], tiles[t][0][:], tiles[t][1][:, 512:1024],
                              start=(t == 0), stop=(t == NT - 1))

                NACT = [[sum(1 for b in bks if b >= m) for m in range(bks[0] + 1)]
                        for bks in HALF_BUCKETS]

                def decode(h, sc_eng):
                    Sb = wpool.tile([128, 512], dt.float32, name=f"Sb{h}")
                    V.tensor_scalar(Sb[:], S[h][:], _f(LN_BIAS), None, Alu.max)
                    Ln2 = wpool.tile([128, 512], dt.float16, name=f"Ln2_{h}")
                    A.activation(Ln2[:], Sb[:].bitcast(dt.int32), Act.Ln,
                                 scale=_f(2.0 ** -23 / K_ENC), bias=lnbias[:])
                    L = wpool.tile([128, 512], dt.float16, name=f"L{h}")
                    for j, b in enumerate(HALF_BUCKETS[h]):
                        inv_s = _f((2 * b + 1) ** 2 / np.float32(18.0))
                        sc_eng.tensor_scalar(L[:, j * 128:(j + 1) * 128],
                                             Ln2[:, j * 128:(j + 1) * 128],
                                             inv_s, None, Alu.mult)
                    return L

                def dt_pyramid(h, src, pass_id, eng):
                    """acc copy + bias planes tmp[m] = src - m*m on `eng`."""
                    bmax = HALF_BUCKETS[h][0]
                    nact = NACT[h]
                    acc = wpool.tile([128, 512], dt.float16, name=f"acc{h}{pass_id}")
                    eng.tensor_copy(acc[:], src[:])
                    tmps = []
                    for m in range(1, bmax + 1):
                        n = nact[m]
                        tm = wpool.tile([128, 512], dt.float16,
                                        name=f"tmp{h}{pass_id}{m}")
                        tmps.append(tm)
                        eng.tensor_scalar(
                            xp3(tm[:], [[128, n], [1, 128]]),
                            xp3(src[:], [[128, n], [1, 128]]),
                            _f(-float(m * m)), None, Alu.add)
                    return acc, tmps

                def dt_chain(h, acc, tmps, on_block_done=None):
                    """Serial shift-max accumulation on DVE (fp16 2x).
                    on_block_done(j) fires once block j's last mag is done."""
                    bks = HALF_BUCKETS[h]
                    bmax = bks[0]
                    nact = NACT[h]
                    for m in range(1, bmax + 1):
                        n = nact[m]
                        tm = tmps[m - 1]
                        wlen = 128 - m
                        pv = xp3(acc[:], [[128, n], [1, wlen]], extra_off=m)
                        tv = xp3(tm[:], [[128, n], [1, wlen]])
                        V.tensor_tensor(pv, pv, tv, Alu.max)
                        nv = xp3(acc[:], [[128, n], [1, wlen]])
                        tv2 = xp3(tm[:], [[128, n], [1, wlen]], extra_off=m)
                        V.tensor_tensor(nv, nv, tv2, Alu.max)
                        if on_block_done is not None:
                            for j, b in enumerate(bks):
                                if b == m:
                                    on_block_done(j)
                    return acc

                def chain_x(h, accx):
                    """pass-x chain with per-block transposes as blocks finish."""
                    acc, tmps = accx
                    Tp = psT.tile([128, 512], dt.float16, name=f"Tp{h}")

                    def xpose(j):
                        PE.transpose(Tp[:, j * 128:(j + 1) * 128],
                                     acc[:, j * 128:(j + 1) * 128], ident16)
                    dt_chain(h, acc, tmps, on_block_done=xpose)
                    L2 = wpool.tile([128, 512], dt.float16, name=f"L2_{h}")
                    V.tensor_copy(L2[:], Tp[:])
                    return L2

                def chain_y(h, accy):
                    """pass-y chain with per-block heat exp as blocks finish."""
                    acc, tmps = accy
                    bks = HALF_BUCKETS[h]
                    H = wpool.tile([128, 512], dt.float16, name=f"H{h}")

                    def heat(j):
                        s_b = _f(np.float32(18.0) / (2 * bks[j] + 1) ** 2)
                        A.activation(H[:, j * 128:(j + 1) * 128],
                                     acc[:, j * 128:(j + 1) * 128],
                                     Act.Exp, scale=s_b)
                    dt_chain(h, acc, tmps, on_block_done=heat)
                    hred = wpool.tile([128, 256], dt.float16, name=f"hred{h}")
                    V.tensor_tensor(hred[:], H[:, 0:256], H[:, 256:512], Alu.max)
                    nc.sync.dma_start(hm_out[h], hred[:])

                # hand-scheduled emission: h1 aux work on Pool rides alongside
                # the DVE-owned h0 chains.
                L0 = decode(0, V)
                a0x = dt_pyramid(0, L0, 0, V)
                L1 = decode(1, G)
                a1x = dt_pyramid(1, L1, 0, G)
                L2_0 = chain_x(0, a0x)
                L2_1 = chain_x(1, a1x)
                a0y = dt_pyramid(0, L2_0, 1, V)
                a1y = dt_pyramid(1, L2_1, 1, G)
                chain_y(0, a0y)
                chain_y(1, a1y)

    nc.compile()
    return nc


def _consts():
    iota1024 = np.arange(1024, dtype=np.float16)
    cst = np.concatenate([
        np.broadcast_to(iota1024, (128, 1024)),
        np.eye(128, dtype=np.float16),
    ], axis=1)
    return np.ascontiguousarray(cst)


def _shard_inputs(refined_rois, refined_scores, medium_gts, medium_scores,
                  near_unmatched, medium_unmatched):
    """Build the 8 per-core input maps (pure layout/sharding, no math)."""
    cst = _consts()
    in_maps = []
    B = refined_rois.shape[0]
    n_rr = refined_rois.shape[1]
    n_nu = near_unmatched.shape[1]
    n_mu = medium_unmatched.shape[1]
    for f in range(B):
        bx = np.concatenate([refined_rois[f][:, :7], medium_gts[f][:, :7],
                             near_unmatched[f][:, :7], medium_unmatched[f][:, :7]], 0)
        score = np.concatenate([refined_scores[f], medium_scores[f],
                                np.full(n_nu, 0.4, np.float32),
                                np.full(n_mu, 0.2, np.float32)])
        cls = np.concatenate([np.full(n_rr, -1.0, np.float32), medium_gts[f][:, 7],
                              np.full(n_nu, -1.0, np.float32),
                              np.full(n_mu, -1.0, np.float32)])
        for hf in range(2):
            sl = slice(hf * NBOX, (hf + 1) * NBOX)

            def lay(a):
                return a[sl].astype(np.float32).reshape(NT, 128).T

            par = np.concatenate([lay(bx[:, 0]), lay(bx[:, 1]), lay(bx[:, 3]),
                                  lay(bx[:, 4]), lay(score), lay(cls)], axis=1)
            in_maps.append(dict(par=np.ascontiguousarray(par), cst=cst))
    return in_maps


def kernel(**inputs) -> np.ndarray:
    from concourse.bass_utils import run_bass_kernel_spmd

    if "nc" not in _prog_cache:
        _prog_cache["nc"] = _build_program()
    nc = _prog_cache["nc"]

    in_maps = _shard_inputs(**{k: np.asarray(v) for k, v in inputs.items()})
    res = run_bass_kernel_spmd(nc, in_maps, core_ids=list(range(8)))
    B = np.asarray(inputs["refined_rois"]).shape[0]
    out = np.empty((B, 1, FEAT, FEAT), np.float32)
    for f in range(B):
        m = None
        for c in (2 * f, 2 * f + 1):
            for k in ("hm0", "hm1"):
                t = res.results[c][k]
                q = np.maximum(t[:, 0:128], t[:, 128:256])
                m = q if m is None else np.maximum(m, q)
        out[f, 0] = m.astype(np.float32).T
    return out
